# revision 1
# baseline (speedup 1.0000x reference)
"""Trainium2 Bass kernel for nn_AttentionBasedModulator.

Computes out[b, n, c, h, w] = query_features[b, c, h, w]
                              * support_fibers[c, n] * diag_weight[c]

Sharding: data-parallel over batch B=8, one batch element per NeuronCore.
Per core: load q[C, H*W] into SBUF (3 tiles of [128, 1024] f32), compute
s[c, n] = support_fibers[c, n] * diag_weight[c] on-chip, then for each
group of prototypes n emit per-partition-scalar multiplies on the DVE
(fp32 tensor_scalar runs in 2x perf mode) into an SBUF tile that is
DMA'd to the contiguous output region. The kernel is DMA-write bound
(~48 MiB of output per core).
"""

import numpy as np

C, NP = 384, 32          # channels, prototypes
B, H, W = 8, 32, 32
HW = H * W
P = 128                  # SBUF partitions
CB = C // P              # channel blocks of 128
N_CORES = 8
N_GROUP = 2              # prototypes per output DMA (tile = N_GROUP*1.5 MiB)
BUFS = 4                 # output tile slots


def build(repeat: int = 1, timing: bool = False, n_group: int = N_GROUP,
          bufs: int = BUFS):
    """Build and compile the Bass program for one core.

    timing=True redirects the big output to an Internal DRAM scratch and
    exposes only a [1, 1] external output, so wall-clock timing of the
    dispatch is not dominated by fetching 400 MB back to the host.
    repeat>1 re-runs the compute+store phase that many times (for marginal
    per-iteration HW timing); the SBUF-resident input load runs once.
    """
    import concourse.bacc as bacc
    import concourse.mybir as mybir
    from concourse.tile import TileContext

    nc = bacc.Bacc(None, target_bir_lowering=False)
    f32 = mybir.dt.float32

    q = nc.dram_tensor("q", [C, HW], f32, kind="ExternalInput")
    sf = nc.dram_tensor("sf", [C, NP], f32, kind="ExternalInput")
    dw = nc.dram_tensor("dw", [C, 1], f32, kind="ExternalInput")
    if timing:
        out = nc.dram_tensor("scratch", [NP, C, HW], f32, kind="Internal")
        tiny = nc.dram_tensor("out", [1, 1], f32, kind="ExternalOutput")
    else:
        out = nc.dram_tensor("out", [NP, C, HW], f32, kind="ExternalOutput")
        tiny = None

    q_r = q.rearrange("(cb p) f -> cb p f", p=P)
    sf_r = sf.rearrange("(cb p) n -> cb p n", p=P)
    dw_r = dw.rearrange("(cb p) o -> cb p o", p=P)
    ng = NP // n_group
    out_r = out.rearrange("(ng g) (cb p) f -> ng p g cb f", p=P, g=n_group)

    with TileContext(nc) as tc:
        with tc.tile_pool(name="consts", bufs=1) as cpool, \
             tc.tile_pool(name="work", bufs=bufs) as wpool:
            q_tiles, s_tiles = [], []
            for cb in range(CB):
                qt = cpool.tile([P, HW], f32, name=f"qt{cb}")
                nc.sync.dma_start(out=qt[:], in_=q_r[cb])
                q_tiles.append(qt)
            for cb in range(CB):
                st = cpool.tile([P, NP], f32, name=f"st{cb}")
                nc.sync.dma_start(out=st[:], in_=sf_r[cb])
                dt_ = cpool.tile([P, 1], f32, name=f"dt{cb}")
                nc.sync.dma_start(out=dt_[:], in_=dw_r[cb])
                nc.vector.tensor_scalar_mul(st[:], st[:], dt_[:])
                s_tiles.append(st)

            for _ in range(repeat):
                for g in range(ng):
                    ot = wpool.tile([P, n_group, CB, HW], f32, name="ot",
                                    tag="ot")
                    for j in range(n_group):
                        n = g * n_group + j
                        for cb in range(CB):
                            nc.vector.tensor_scalar_mul(
                                ot[:, j, cb, :], q_tiles[cb][:],
                                s_tiles[cb][:, n:n + 1])
                    nc.sync.dma_start(out=out_r[g], in_=ot[:])

            if timing:
                tt = wpool.tile([1, 1], f32, name="tt", tag="tt")
                nc.vector.tensor_copy(out=tt[:], in_=s_tiles[0][:1, :1])
                nc.sync.dma_start(out=tiny[:], in_=tt[:])

    nc.compile()
    return nc


def make_in_maps(support_fibers, query_features, diag_weight):
    qf = np.ascontiguousarray(
        np.asarray(query_features, dtype=np.float32).reshape(B, C, HW))
    sfm = np.ascontiguousarray(np.asarray(support_fibers, dtype=np.float32))
    dwm = np.ascontiguousarray(
        np.asarray(diag_weight, dtype=np.float32).reshape(C, 1))
    return [{"q": qf[b], "sf": sfm, "dw": dwm} for b in range(B)]


_nc_cache = None


def kernel(support_fibers, query_features, diag_weight):
    from concourse.bass_utils import run_bass_kernel_spmd

    global _nc_cache
    if _nc_cache is None:
        _nc_cache = build()

    in_maps = make_in_maps(support_fibers, query_features, diag_weight)
    res = run_bass_kernel_spmd(_nc_cache, in_maps, core_ids=list(range(N_CORES)))
    out = np.stack([res.results[b]["out"] for b in range(B)], axis=0)
    return out.reshape(B, NP, C, H, W)


# revision 2
# speedup vs baseline: 1.1980x; 1.1980x over previous
"""Trainium2 Bass kernel for nn_AttentionBasedModulator.

Computes out[b, n, c, h, w] = query_features[b, c, h, w]
                              * support_fibers[c, n] * diag_weight[c]

Sharding: data-parallel over batch B=8, one batch element per NeuronCore.
Per core: load q[C, H*W] into SBUF (3 tiles of [128, 1024] f32), compute
s[c, n] = support_fibers[c, n] * diag_weight[c] on-chip, then for each
group of prototypes n emit per-partition-scalar multiplies (fp32
tensor_scalar runs in 2x DVE perf mode; optionally a slice of the work
goes to the scalar/ACT engine) into an SBUF tile that is DMA'd to the
contiguous output region. The kernel is DMA-write bound (~48 MiB of
output per core).
"""

import numpy as np

C, NP = 384, 32          # channels, prototypes
B, H, W = 8, 32, 32
HW = H * W
P = 128                  # SBUF partitions
CB = C // P              # channel blocks of 128
N_CORES = 8
N_GROUP = 2              # prototypes per output DMA (tile = N_GROUP*1.5 MiB)
BUFS = 4                 # output tile slots
ACT_SPLIT = 0            # of each group's n_group*CB multiplies, how many go
                         # to the ACT (scalar) engine instead of the DVE


def build(repeat: int = 1, timing: bool = False, n_group: int = N_GROUP,
          bufs: int = BUFS, act_split: int = ACT_SPLIT, dma_lite: bool = False):
    """Build and compile the Bass program for one core.

    timing=True: each repeat writes a distinct Internal DRAM region (so
    stores cannot be dead-store-eliminated); a final DRAM->DRAM readback
    of a few bytes per region forms the only ExternalOutput, so dispatch
    timing is not dominated by fetching 400 MB to the host.
    dma_lite=True: only one multiply per output tile (rest of the tile is
    stale slot data) - isolates DMA-write throughput from DVE work.
    """
    import concourse.bacc as bacc
    import concourse.mybir as mybir
    from concourse.tile import TileContext

    nc = bacc.Bacc(None, target_bir_lowering=False)
    f32 = mybir.dt.float32
    act_copy = mybir.ActivationFunctionType.Copy

    q = nc.dram_tensor("q", [C, HW], f32, kind="ExternalInput")
    sf = nc.dram_tensor("sf", [C, NP], f32, kind="ExternalInput")
    dw = nc.dram_tensor("dw", [C, 1], f32, kind="ExternalInput")
    if timing:
        scratch = nc.dram_tensor("scratch", [repeat, NP, C, HW], f32,
                                 kind="Internal")
        tiny = nc.dram_tensor("out", [repeat, 4], f32, kind="ExternalOutput")
        out_views = [scratch[r] for r in range(repeat)]
    else:
        out = nc.dram_tensor("out", [NP, C, HW], f32, kind="ExternalOutput")
        tiny = None
        out_views = [out] * repeat

    q_r = q.rearrange("(cb p) f -> cb p f", p=P)
    sf_r = sf.rearrange("(cb p) n -> cb p n", p=P)
    dw_r = dw.rearrange("(cb p) o -> cb p o", p=P)
    ng = NP // n_group

    with TileContext(nc) as tc:
        with tc.tile_pool(name="consts", bufs=1) as cpool, \
             tc.tile_pool(name="work", bufs=bufs) as wpool:
            q_tiles, s_tiles = [], []
            for cb in range(CB):
                qt = cpool.tile([P, HW], f32, name=f"qt{cb}")
                nc.sync.dma_start(out=qt[:], in_=q_r[cb])
                q_tiles.append(qt)
            for cb in range(CB):
                st = cpool.tile([P, NP], f32, name=f"st{cb}")
                nc.sync.dma_start(out=st[:], in_=sf_r[cb])
                dt_ = cpool.tile([P, 1], f32, name=f"dt{cb}")
                nc.sync.dma_start(out=dt_[:], in_=dw_r[cb])
                nc.vector.tensor_scalar_mul(st[:], st[:], dt_[:])
                s_tiles.append(st)

            for r in range(repeat):
                out_r = out_views[r].rearrange(
                    "(ng g) (cb p) f -> ng p g cb f", p=P, g=n_group)
                for g in range(ng):
                    ot = wpool.tile([P, n_group, CB, HW], f32, name="ot",
                                    tag="ot")
                    k = 0
                    for j in range(n_group):
                        n = g * n_group + j
                        for cb in range(CB):
                            if dma_lite and k > 0:
                                k += 1
                                continue
                            if k < act_split:
                                nc.scalar.activation(
                                    ot[:, j, cb, :], q_tiles[cb][:], act_copy,
                                    scale=s_tiles[cb][:, n:n + 1])
                            else:
                                nc.vector.tensor_scalar_mul(
                                    ot[:, j, cb, :], q_tiles[cb][:],
                                    s_tiles[cb][:, n:n + 1])
                            k += 1
                    nc.sync.dma_start(out=out_r[g], in_=ot[:])

            if timing:
                nc.sync.dma_start(out=tiny[:], in_=scratch[:, 0, 0, 0:4])

    nc.compile()
    return nc


def make_in_maps(support_fibers, query_features, diag_weight):
    qf = np.ascontiguousarray(
        np.asarray(query_features, dtype=np.float32).reshape(B, C, HW))
    sfm = np.ascontiguousarray(np.asarray(support_fibers, dtype=np.float32))
    dwm = np.ascontiguousarray(
        np.asarray(diag_weight, dtype=np.float32).reshape(C, 1))
    return [{"q": qf[b], "sf": sfm, "dw": dwm} for b in range(B)]


_nc_cache = None


def kernel(support_fibers, query_features, diag_weight):
    from concourse.bass_utils import run_bass_kernel_spmd

    global _nc_cache
    if _nc_cache is None:
        _nc_cache = build()

    in_maps = make_in_maps(support_fibers, query_features, diag_weight)
    res = run_bass_kernel_spmd(_nc_cache, in_maps, core_ids=list(range(N_CORES)))
    out = np.stack([res.results[b]["out"] for b in range(B)], axis=0)
    return out.reshape(B, NP, C, H, W)
